# revision 39
# baseline (speedup 1.0000x reference)
"""DREAM rank-R fast-weight recurrence kernel for Trainium2 (8 NeuronCores).

Strategy (data-parallel over batch, 8 samples/core):
  Phase 1: transpose x -> x.T tiles (PE transposes); bf16 + fp32 copies.
  Phase A: v = x @ W_v  (big bf16 matmuls) -> DRAM.
  Phase B: chunked linear attention (chunk C=128) producing
           z = x@W_x + b + fast  (fast = decayed rank-R readout) -> DRAM fp32.
           x@W_x runs in float32r (tf32-ish) — same 1 cycle/row rate as bf16
           at N=512 but ~0.05% error instead of 0.4%.
  Phase C: sequential scan over T=512: h_t = tanh(z_t + h_{t-1} @ W_h),
           software-pipelined: body(iv) = [post-process step iv from psum
           slot iv%2 (tanh -> PE transpose -> pack), interleaved quarter-by-
           quarter with the matmul phase of step iv+1 into slot (iv+1)%2].
           This keeps the PE streaming continuously (HAM stays warm) and
           overlaps tanh/transpose/pack of one step with matmuls of the next.
Output written as (T, B, H) fp32; host just transposes to (B, T, H).
"""

import numpy as np

LAST_EXEC_NS = None

import concourse.bass as bass
import concourse.mybir as mybir
import concourse.tile as tile
from concourse.bass import _add_dep_helper
from concourse.masks import make_identity, make_upper_triangular

FP = mybir.dt.float32
FR = mybir.dt.float32r
BF = mybir.dt.bfloat16
ACTF = mybir.ActivationFunctionType
ALU = mybir.AluOpType

# full-problem config
B_FULL, T, I, H, R = 64, 512, 1024, 2048, 8
N_CORES = 8
SCAN_FR = False  # h @ W_h in float32r (else bf16; fp32r can't target offset
                 # psum partitions — s3d3_mm_valid_dst_partition)
ZX_FR = True     # x @ W_x in float32r (else bf16)


def _squeeze0(ap):
    # drop a leading size-1 dim from an AP via rearrange
    shape = ap.shape
    assert shape[0] == 1
    letters = list("abcdefg"[: len(shape)])
    lhs = " ".join(letters)
    rhs = f"({letters[0]} {letters[1]})" + ("" if len(letters) < 3 else " " + " ".join(letters[2:]))
    return ap.rearrange(f"{lhs} -> {rhs}")


def split_excess_waits(nc, max_waits=1):
    """walrus in this env supports only 1 sync-wait per instruction; split
    extras onto standalone NoOps on the same engine (order-preserving)."""
    n = 0
    for fn in nc.m.functions:
        for bb in fn.blocks:
            new_insts = []
            for inst in bb.instructions:
                si = getattr(inst, "sync_info", None)
                if si is not None and si.on_wait and len(si.on_wait) > max_waits:
                    waits = list(si.on_wait)
                    extra, keep = waits[:-max_waits], waits[-max_waits:]
                    for w in extra:
                        new_insts.append(mybir.InstNoOp(
                            name=f"waitsplit_{nc.next_id()}",
                            engine=inst.engine, ins=[], outs=[],
                            bass_nofuse=True,
                            sync_info=mybir.SyncInfo(on_wait=[w], on_update=[]),
                        ))
                        n += 1
                    inst.sync_info = mybir.SyncInfo(on_wait=keep, on_update=list(si.on_update))
                new_insts.append(inst)
            bb.instructions = new_insts
    return n


def build_kernel(nc, B=8, T=T, I=I, H=H, R=R, unroll=8):
    """Emit the full per-core program. B = per-core batch."""
    _chain = {}

    def _dep(tag, inst):
        cur_bb = nc.cur_bb.bb.name if nc.cur_bb is not None else None
        prev = _chain.get(tag)
        if prev is not None and prev[0] == cur_bb:
            _add_dep_helper(inst.ins, prev[1].ins, sync=False, reason="psum chain")
        _chain[tag] = (cur_bb, inst)
        return inst

    def cmm(tag, *a, **kw):
        return _dep(tag, nc.tensor.matmul(*a, **kw))

    def ctr(tag, *a, **kw):
        return _dep(tag, nc.tensor.transpose(*a, **kw))

    KI, KH = I // 128, H // 128
    NS = max(1, H // 512)          # 512-wide column slices of H
    assert H % 512 == 0
    NTOK = B * T
    NC_CH = T // 128               # chunks (chunk == 128-token window)
    assert B <= 8 and T % 128 == 0 and I % 128 == 0
    NQ = (B + 3) // 4              # F state tiles (4 batches per 128-partition tile)

    x_ext = nc.declare_dram_parameter("x", [B, T, I], FP, isOutput=False)
    Wx_ext = nc.declare_dram_parameter("W_x", [I, H], FP, isOutput=False)
    Wh_ext = nc.declare_dram_parameter("W_h", [H, H], FP, isOutput=False)
    Wk_ext = nc.declare_dram_parameter("W_k", [I, R], FP, isOutput=False)
    Wv_ext = nc.declare_dram_parameter("W_v", [I, H], FP, isOutput=False)
    Wl_ext = nc.declare_dram_parameter("W_lam", [I, R], FP, isOutput=False)
    b_ext = nc.declare_dram_parameter("b", [H], FP, isOutput=False)
    bl_ext = nc.declare_dram_parameter("b_lam", [R], FP, isOutput=False)
    out_ext = nc.declare_dram_parameter("out", [T, 128, 512], FP, isOutput=True)

    v_dram = nc.dram_tensor("v_dram", [NTOK, H], BF)
    # z stored as two-term bf16 (hi at [0:H], lo residual at [H:2H]) so the
    # scan's identity-matmul injection stays bf16 while z keeps ~fp32
    # precision. Step dim padded so zbuf prefetch may read 2 steps past T.
    z_dram = nc.dram_tensor("z_dram", [B, T + 8, 2 * H], BF)

    ZXD = FR if ZX_FR else BF      # dtype of the x@W_x matmuls
    SCD = FR if SCAN_FR else BF    # dtype of the scan matmuls

    with tile.TileContext(nc) as tc:
        # ---------- persistent pool ----------
        with tc.tile_pool(name="persist", bufs=1) as pp:
            I128 = pp.tile([128, 128], FP)
            make_identity(nc, I128[:])
            I128b = pp.tile([128, 128], BF)
            nc.vector.tensor_copy(I128b[:], I128[:])
            I8rep = pp.tile([128, max(B, 1)], FP)
            nc.gpsimd.memset(I8rep[:], 0.0)
            for g in range(4):
                nc.gpsimd.affine_select(
                    out=I8rep[:], in_=I8rep[:], compare_op=ALU.not_equal,
                    fill=1.0, base=-32 * g, pattern=[[-1, B]], channel_multiplier=1)
            IB_bf = pp.tile([B, B], BF)
            nc.vector.tensor_copy(IB_bf[:], I8rep[0:B, 0:B])
            # interleaved double identity [2B, B] for one-shot hi+lo z
            # injection: rows 2j and 2j+1 both map to output row j
            IB2_f = pp.tile([2 * B, B], FP)
            nc.gpsimd.memset(IB2_f[:], 0.0)
            for off in range(2):
                nc.gpsimd.affine_select(
                    out=IB2_f[:], in_=IB2_f[:], compare_op=ALU.not_equal,
                    fill=1.0, base=-off, pattern=[[-2, B]], channel_multiplier=1)
            IB2_bf = pp.tile([2 * B, B], BF)
            nc.vector.tensor_copy(IB2_bf[:], IB2_f[:])
            Tri = pp.tile([128, 128], FP)       # Tri[u,t] = u <= t
            make_upper_triangular(nc, Tri[:], val=1.0, diag=True)
            STri = pp.tile([128, 128], FP)      # STri[u,t] = u < t
            make_upper_triangular(nc, STri[:], val=1.0, diag=False)

            b_bcast = pp.tile([128, H], FP)
            nc.sync.dma_start(out=b_bcast[:], in_=b_ext[None, :].to_broadcast((128, H)))
            bl_bcast = pp.tile([128, R], FP)
            nc.sync.dma_start(out=bl_bcast[:], in_=bl_ext[None, :].to_broadcast((128, R)))

            Wk_bf = pp.tile([128, KI, R], BF)
            Wl_bf = pp.tile([128, KI, R], BF)

            def load_xT(j, w, pool, pspool, tagp, want32=False):
                # DMA an x window (128 tok, I) and transpose to [128, KI, 128]
                xs = pool.tile([128, I], FP, name=f"xs_{tagp}_{j}_{w}", tag=f"xs_{tagp}", bufs=2)
                nc.sync.dma_start(out=xs[:], in_=x_ext[j, w * 128:(w + 1) * 128, :])
                xTw = pool.tile([128, KI, 128], BF, name=f"xTw_{tagp}_{j}_{w}", tag=f"xTw_{tagp}", bufs=2)
                xT32 = None
                if want32:
                    xT32 = pool.tile([128, KI, 128], FP, name=f"xT32_{tagp}_{j}_{w}", tag=f"xT32_{tagp}", bufs=2)
                for ki in range(KI):
                    xt_ps = pspool.tile([128, 128], FP, name=f"xtps_{tagp}_{j}_{w}_{ki}", tag=f"xtps_{tagp}", bufs=1)
                    ctr(f"xtps_{tagp}", xt_ps[:], xs[:, ki * 128:(ki + 1) * 128], I128[:])
                    nc.vector.tensor_copy(xTw[:, ki, :], xt_ps[:])
                    if want32:
                        nc.scalar.copy(xT32[:, ki, :].bitcast(FR), xt_ps[:])
                return xTw, xT32

            with tc.tile_pool(name="wstage", bufs=2) as wsp:
                for ki in range(KI):
                    wkst = wsp.tile([128, R], FP, name=f"wkst{ki}", tag="wkst")
                    nc.sync.dma_start(out=wkst[:], in_=Wk_ext[ki * 128:(ki + 1) * 128, :])
                    nc.vector.tensor_copy(Wk_bf[:, ki, :], wkst[:])
                    wlst = wsp.tile([128, R], FP, name=f"wlst{ki}", tag="wlst")
                    nc.sync.dma_start(out=wlst[:], in_=Wl_ext[ki * 128:(ki + 1) * 128, :])
                    nc.vector.tensor_copy(Wl_bf[:, ki, :], wlst[:])

            # ---------- Phase A: v = x @ W_v ----------
            with tc.tile_pool(name="phA", bufs=2) as pA, \
                 tc.tile_pool(name="phAps", bufs=2, space="PSUM") as pAps:
                Wv_bf = pA.tile([128, KI, H], BF, bufs=1)
                for ki in range(KI):
                    wst = pA.tile([128, H], FP, name=f"wvst{ki}", tag="wvst", bufs=1)
                    nc.sync.dma_start(out=wst[:], in_=Wv_ext[ki * 128:(ki + 1) * 128, :])
                    nc.vector.tensor_copy(Wv_bf[:, ki, :], wst[:])
                for j in range(B):
                    for w in range(NC_CH):
                        tok0 = j * T + w * 128
                        xTw, _ = load_xT(j, w, pA, pAps, "A")
                        # ki-outer so the 4 column slices share each weight load
                        v_pss = [pAps.tile([128, 512], FP, name=f"vps{j}_{w}_{s}",
                                           tag=f"vps{s}", bufs=1) for s in range(NS)]
                        for ki in range(KI):
                            for s in range(NS):
                                cmm(f"vps{s}", v_pss[s][:], xTw[:, ki, :],
                                    Wv_bf[:, ki, s * 512:(s + 1) * 512],
                                    start=(ki == 0), stop=(ki == KI - 1))
                        for s in range(NS):
                            v_sb = pA.tile([128, 512], BF, name=f"vsb{j}_{w}_{s}", tag="vsb")
                            nc.vector.tensor_copy(v_sb[:], v_pss[s][:])
                            nc.sync.dma_start(
                                out=v_dram[tok0:tok0 + 128, s * 512:(s + 1) * 512], in_=v_sb[:])

            # ---------- Phase B: z = x@W_x + b + fast ----------
            with tc.tile_pool(name="phB", bufs=2) as pB, \
                 tc.tile_pool(name="phBst", bufs=1) as pBst, \
                 tc.tile_pool(name="phBps", bufs=2, space="PSUM") as pBps:
                if ZX_FR:
                    Wx_st = pBst.tile([128, KI, H], FP)
                    for ki in range(KI):
                        nc.sync.dma_start(out=Wx_st[:, ki, :].bitcast(FR),
                                          in_=Wx_ext[ki * 128:(ki + 1) * 128, :].bitcast(FR))
                else:
                    Wx_st = pBst.tile([128, KI, H], BF)
                    for ki in range(KI):
                        wst2 = pB.tile([128, H], FP, name=f"wxst{ki}", tag="wxst", bufs=1)
                        nc.sync.dma_start(out=wst2[:], in_=Wx_ext[ki * 128:(ki + 1) * 128, :])
                        nc.vector.tensor_copy(Wx_st[:, ki, :], wst2[:])
                F_T = [pBst.tile([128, H], FP, name=f"FT{q}") for q in range(NQ)]
                F_bfT = [pBst.tile([128, H], BF, name=f"FbfT{q}") for q in range(NQ)]
                for q in range(NQ):
                    nc.gpsimd.memset(F_T[q][:], 0.0)
                    nc.gpsimd.memset(F_bfT[q][:], 0.0)

                for c in range(NC_CH):
                    cc_all = [pB.tile([128, 1], FP, name=f"cc{c}_{q}", tag=f"cc{q}")
                              for q in range(NQ)]
                    for q in range(NQ):
                        nc.gpsimd.memset(cc_all[q][:], 0.0)
                    b_bfs, v_sbs = [], []
                    for j in range(B):
                        q, m = j // 4, j % 4
                        tok0 = j * T + c * 128
                        # -- stage v chunk
                        v_sb = pB.tile([128, H], BF, name=f"vst{c}_{j}", tag=f"vst{j}", bufs=1)
                        nc.sync.dma_start(out=v_sb[:], in_=v_dram[tok0:tok0 + 128, :])
                        v_sbs.append(v_sb)
                        xTw, xT32 = load_xT(j, c, pB, pBps, "B", want32=ZX_FR)
                        # -- k, lam projections (psum)
                        sm = pBps.tile([128, 512], FP, name=f"small{c}_{j}", tag="small", bufs=2)
                        k_ps, l_ps = sm[:, 0:R], sm[:, 16:16 + R]
                        Lc_ps, Lcp_ps = sm[:, 32:32 + R], sm[:, 48:48 + R]
                        aT_ps, bT_ps, LcT_ps = sm[:, 64:192], sm[:, 192:320], sm[:, 320:448]
                        for ki in range(KI):
                            cmm("small", k_ps, xTw[:, ki, :],
                                Wk_bf[:, ki, :], start=(ki == 0), stop=(ki == KI - 1))
                        for ki in range(KI):
                            cmm("small", l_ps, xTw[:, ki, :],
                                Wl_bf[:, ki, :], start=(ki == 0), stop=(ki == KI - 1))
                        # -- k = elu(.)+1 normalized
                        r1 = pB.tile([128, R], FP, name=f"r1{c}{j}", tag="r1")
                        nc.scalar.activation(r1[:], k_ps, ACTF.Relu)
                        r2 = pB.tile([128, R], FP, name=f"r2{c}{j}", tag="r2")
                        nc.vector.tensor_scalar_min(r2[:], k_ps, 0.0)
                        r3 = pB.tile([128, R], FP, name=f"r3{c}{j}", tag="r3")
                        nc.scalar.activation(r3[:], r2[:], ACTF.Exp)
                        ku = pB.tile([128, R], FP, name=f"ku{c}{j}", tag="ku")
                        nc.vector.tensor_tensor(ku[:], r1[:], r3[:], op=ALU.add)
                        ksum = pB.tile([128, 1], FP, name=f"ksum{c}{j}", tag="ksum")
                        nc.vector.reduce_sum(ksum[:], ku[:], axis=mybir.AxisListType.X)
                        kinv = pB.tile([128, 1], FP, name=f"kinv{c}{j}", tag="kinv")
                        nc.vector.reciprocal(kinv[:], ksum[:])
                        k_sb = pB.tile([128, R], FP, name=f"ksb{c}{j}", tag="ksb")
                        nc.vector.tensor_scalar_mul(k_sb[:], ku[:], kinv[:])
                        # -- loglam = -ln(1 + exp(-(zlam + b_lam)))
                        zl = pB.tile([128, R], FP, name=f"zl{c}{j}", tag="zl")
                        nc.vector.tensor_tensor(zl[:], l_ps, bl_bcast[:, :], op=ALU.add)
                        el = pB.tile([128, R], FP, name=f"el{c}{j}", tag="el")
                        nc.scalar.activation(el[:], zl[:], ACTF.Exp, scale=-1.0)
                        ep1 = pB.tile([128, R], FP, name=f"ep1{c}{j}", tag="ep1")
                        nc.vector.tensor_scalar_add(ep1[:], el[:], 1.0)
                        lsp = pB.tile([128, R], FP, name=f"lsp{c}{j}", tag="lsp")
                        nc.scalar.activation(lsp[:], ep1[:], ACTF.Ln)
                        ll = pB.tile([128, R], FP, name=f"ll{c}{j}", tag="ll")
                        nc.vector.tensor_scalar_mul(ll[:], lsp[:], -1.0)
                        # -- cumsums over time (within chunk) via triangular matmuls
                        cmm("small", Lc_ps, Tri[:], ll[:], start=True, stop=True)
                        cmm("small", Lcp_ps, STri[:], ll[:], start=True, stop=True)
                        cprev = pB.tile([128, R], FP, name=f"cprev{c}{j}", tag="cprev")
                        nc.scalar.activation(cprev[:], Lcp_ps, ACTF.Exp)
                        cinv = pB.tile([128, R], FP, name=f"cinv{c}{j}", tag="cinv")
                        nc.scalar.activation(cinv[:], Lc_ps, ACTF.Exp, scale=-1.0)
                        a_sb = pB.tile([128, R], FP, name=f"asb{c}{j}", tag="asb")
                        nc.vector.tensor_tensor(a_sb[:], k_sb[:], cprev[:], op=ALU.mult)
                        b_sb = pB.tile([128, R], FP, name=f"bsb{c}{j}", tag="bsb")
                        nc.vector.tensor_tensor(b_sb[:], k_sb[:], cinv[:], op=ALU.mult)
                        b_bf = pB.tile([128, R], BF, name=f"bbf{c}_{j}", tag=f"bbf{j}", bufs=2)
                        nc.vector.tensor_copy(b_bf[:], b_sb[:])
                        b_bfs.append(b_bf)
                        Lc_sb = pB.tile([128, R], FP, name=f"lcsb{c}{j}", tag="lcsb")
                        nc.vector.tensor_copy(Lc_sb[:], Lc_ps)
                        # -- transposes to (R, 128) at base 0 (walrus: transpose out base must be 0)
                        ctr("small", aT_ps[0:R, :], a_sb[:], I128[:])
                        ctr("small", bT_ps[0:R, :], b_sb[:], I128[:])
                        ctr("small", LcT_ps[0:R, :], Lc_sb[:], I128[:])
                        aT_bf = pB.tile([R, 128], BF, name=f"aT{c}{j}", tag="aT")
                        nc.vector.tensor_copy(aT_bf[:], aT_ps[0:R, :])
                        bT_bf = pB.tile([R, 128], BF, name=f"bT{c}{j}", tag="bT")
                        nc.vector.tensor_copy(bT_bf[:], bT_ps[0:R, :])
                        # replicate a.T to base 32m for the inter matmul (DMA crosses partitions)
                        aT32_bf = pB.tile([128, 128], BF, name=f"aT32_{c}{j}", tag="aT32")
                        nc.sync.dma_start(out=aT32_bf[32 * m:32 * m + R, :], in_=aT_bf[:])
                        cc0 = pB.tile([R, 1], FP, name=f"cc0{c}{j}", tag="cc0")
                        nc.scalar.activation(cc0[:], LcT_ps[0:R, 127:128], ACTF.Exp)
                        nc.sync.dma_start(out=cc_all[q][32 * m:32 * m + R, :], in_=cc0[:])
                        # -- S.T = mask(b @ a.T) (u,t), bf16
                        ST_ps = pBps.tile([128, 128], FP, name=f"stps{c}{j}", tag="stps", bufs=1)
                        cmm("stps", ST_ps[:], bT_bf[:], aT_bf[:], start=True, stop=True)
                        ST_sb = pB.tile([128, 128], BF, name=f"stsb{c}{j}", tag="stsb")
                        nc.vector.tensor_tensor(ST_sb[:], ST_ps[:], STri[:], op=ALU.mult)
                        # -- z tile: x@W_x + intra + inter, + bias on evac
                        for s in range(NS):
                            z_ps = pBps.tile([128, 512], FP, name=f"zps{c}_{j}_{s}", tag="zps", bufs=2)
                            for ki in range(KI):
                                if ZX_FR:
                                    cmm("zps", z_ps[:], xT32[:, ki, :].bitcast(FR),
                                        Wx_st[:, ki, s * 512:(s + 1) * 512].bitcast(FR),
                                        start=(ki == 0), stop=False)
                                else:
                                    cmm("zps", z_ps[:], xTw[:, ki, :],
                                        Wx_st[:, ki, s * 512:(s + 1) * 512],
                                        start=(ki == 0), stop=False)
                            cmm("zps", z_ps[:], ST_sb[:], v_sb[:, s * 512:(s + 1) * 512],
                                start=False, stop=(c == 0))
                            if c > 0:
                                cmm("zps", z_ps[:], aT32_bf[32 * m:32 * m + R, :],
                                    F_bfT[q][32 * m:32 * m + R, s * 512:(s + 1) * 512],
                                    start=False, stop=True,
                                    tile_position=(32 * m, 0))
                            z_sb = pB.tile([128, 512], FP, name=f"zsb{c}_{j}_{s}", tag="zsb")
                            nc.vector.tensor_tensor(z_sb[:], z_ps[:],
                                                    b_bcast[:, s * 512:(s + 1) * 512], op=ALU.add)
                            z_hi = pB.tile([128, 512], BF, name=f"zhi{c}_{j}_{s}", tag="zhi")
                            nc.scalar.copy(z_hi[:], z_sb[:])
                            z_lo = pB.tile([128, 512], BF, name=f"zlo{c}_{j}_{s}", tag="zlo")
                            nc.vector.tensor_tensor(z_lo[:], z_sb[:], z_hi[:],
                                                    op=ALU.subtract)
                            rows = slice(c * 128, (c + 1) * 128)
                            nc.sync.dma_start(
                                out=z_dram[j, rows, s * 512:(s + 1) * 512], in_=z_hi[:])
                            nc.sync.dma_start(
                                out=z_dram[j, rows, H + s * 512:H + (s + 1) * 512], in_=z_lo[:])
                    # -- F state update: F = (F + sum_u b_u v_u^T) * cC
                    for s in range(NS):
                        Fd_ps = [pBps.tile([128, 512], FP, name=f"fd{c}_{s}_{q}", tag=f"fd{q}", bufs=1)
                                 for q in range(NQ)]
                        for q in range(NQ):
                            nc.vector.memset(Fd_ps[q][:], 0.0)
                        for j in range(B):
                            q, m = j // 4, j % 4
                            cmm(f"fd{q}", Fd_ps[q][32 * m:32 * m + R, :], b_bfs[j][:],
                                v_sbs[j][:, s * 512:(s + 1) * 512],
                                start=True, stop=True,
                                tile_position=(0, 32 * m))
                        for q in range(NQ):
                            sl = slice(s * 512, (s + 1) * 512)
                            nc.vector.tensor_tensor(F_T[q][:, sl], F_T[q][:, sl], Fd_ps[q][:],
                                                    op=ALU.add)
                            nc.vector.tensor_scalar_mul(F_T[q][:, sl], F_T[q][:, sl], cc_all[q][:])
                            nc.scalar.copy(F_bfT[q][:, sl], F_T[q][:, sl])

            # ---------- Phase C: the scan (software-pipelined) ----------
            with tc.tile_pool(name="phC", bufs=1) as pC, \
                 tc.tile_pool(name="phCps", bufs=1, space="PSUM") as pCps:
                HD = FP if SCAN_FR else BF      # h storage dtype
                if SCAN_FR:
                    Wh_st = pC.tile([128, KH, H], FP)
                    for kh in range(KH):
                        nc.sync.dma_start(out=Wh_st[:, kh, :].bitcast(FR),
                                          in_=Wh_ext[kh * 128:(kh + 1) * 128, :].bitcast(FR))
                else:
                    Wh_st = pC.tile([128, KH, H], BF)
                    with tc.tile_pool(name="whload", bufs=2) as wl:
                        for kh in range(KH):
                            wst3 = wl.tile([128, H], FP, name=f"whst{kh}", tag="whst")
                            nc.sync.dma_start(out=wst3[:], in_=Wh_ext[kh * 128:(kh + 1) * 128, :])
                            nc.vector.tensor_copy(Wh_st[:, kh, :], wst3[:])

                # packed transposed h: [128, 4(b)*4(a)*8(j)]; lhsT for tile
                # kh=(a,b) is h_T[:, 32b+8a : 32b+8a+8]
                assert not SCAN_FR
                h_T = pC.tile([128, 128], HD)
                nc.gpsimd.memset(h_T[:], 0.0)
                ps1 = [pCps.tile([128, 512], FP, name=f"ps1_{i}") for i in range(2)]
                ps2 = [pCps.tile([128, 128], HD, name=f"ps2_{i}") for i in range(2)]
                for i in range(2):
                    nc.vector.memset(ps1[i][:], 0.0)
                pre_t = [pC.tile([128, 512], HD, name=f"pret{i}") for i in range(2)]
                hout = [pC.tile([128, 512], FP, name=f"hout{i}") for i in range(2)]
                zbuf = [pC.tile([B, 2 * H], BF, name=f"zbuf{i}") for i in range(4)]

                def inject(slot, zb, stop=False):
                    # add z (hi + lo bf16 terms) into ps1[slot] via identity
                    # matmuls; hi terms start the accumulation group
                    for s in range(NS):
                        cmm("ps1", ps1[slot][32 * s:32 * s + B, :],
                            IB_bf[:], zb[:, 512 * s:512 * (s + 1)],
                            start=True, stop=False,
                            tile_position=(0, 32 * s), skip_group_check=True)
                    for s in range(NS):
                        cmm("ps1", ps1[slot][32 * s:32 * s + B, :],
                            IB_bf[:], zb[:, H + 512 * s:H + 512 * (s + 1)],
                            start=False, stop=stop and (s == NS - 1),
                            tile_position=(0, 32 * s), skip_group_check=True)

                def zdma(zb, iv_expr):
                    nc.sync.dma_start(
                        out=zb[:],
                        in_=_squeeze0(z_dram[0:B, iv_expr, :].rearrange("b one h -> one b h")))

                # prologue: prefetch z for steps 0/1, inject step 0 into slot 0
                zdma(zbuf[0], slice(0, 1))
                zdma(zbuf[1], slice(1, 2))
                inject(0, zbuf[0], stop=True)

                kcnt = [0]

                def step(iv):
                    k = kcnt[0] % unroll
                    kcnt[0] += 1
                    sl, nx = k % 2, (k + 1) % 2
                    # prefetch z for step iv+2
                    zdma(zbuf[(k + 2) % 4], bass.ds(iv + 2, 1))
                    # start accumulating step iv+1: z first
                    inject(nx, zbuf[(k + 1) % 4])
                    # post(iv) tanh quarters first (ACT runs them in order)
                    for b4 in range(4):
                        nc.scalar.activation(pre_t[sl][:, 128 * b4:128 * (b4 + 1)],
                                             ps1[sl][:, 128 * b4:128 * (b4 + 1)], ACTF.Tanh)

                    def trans(b4):
                        # PE transpose of quarter b4, streaming only the 32
                        # identity columns for valid (a, j) lanes, so the psum
                        # result lands pre-packed; DVE copy is then contiguous
                        ctr("ps2", ps2[sl][:, 32 * b4:32 * (b4 + 1)],
                            pre_t[sl][:, 128 * b4:128 * (b4 + 1)],
                            I128b[:].rearrange("p (a q) -> p a q", q=32)[:, :, 0:B])
                        nc.vector.tensor_copy(
                            h_T[:, 32 * b4:32 * (b4 + 1)],
                            ps2[sl][:, 32 * b4:32 * (b4 + 1)])

                    def burst(b4):
                        # matmul burst for quarter b4 of step iv+1
                        for a in range(4):
                            kh = 4 * a + b4
                            lhs = h_T[:, 32 * b4 + 8 * a:32 * b4 + 8 * a + 8]
                            for s in range(NS):
                                cmm("ps1", ps1[nx][32 * s:32 * s + B, :],
                                    lhs, Wh_st[:, kh, 512 * s:512 * (s + 1)],
                                    start=False, stop=(b4 == 3 and a == 3),
                                    tile_position=(0, 32 * s), skip_group_check=True)

                    # lookahead: PE does trans(b+1) while DVE packs quarter b,
                    # so each burst finds its lhsT ready without PE idle
                    trans(0)
                    for b4 in range(1, 4):
                        trans(b4)
                        burst(b4 - 1)
                    burst(3)
                    # fp32 output for step iv (off the critical path)
                    # fp32 output (garbage lanes included; host slices the
                    # valid 8-row band of each 32-partition group)
                    nc.scalar.activation(hout[sl][:], ps1[sl][:], ACTF.Tanh)
                    nc.sync.dma_start(out=_squeeze0(out_ext[bass.ds(iv, 1)]),
                                      in_=hout[sl][:])

                tc.For_i_unrolled(0, T, 1, step, max_unroll=unroll)

    return nc


# ---------------- host-side wrapper ----------------

def kernel(**inputs):
    from concourse.bass_utils import run_bass_kernel_spmd
    x = np.asarray(inputs["x"], np.float32)
    B, Tl, Il = x.shape
    Hl = np.asarray(inputs["W_x"]).shape[1]
    Rl = np.asarray(inputs["W_k"]).shape[1]
    Bl = B // N_CORES

    nc = bass.Bass()
    build_kernel(nc, B=Bl, T=Tl, I=Il, H=Hl, R=Rl)
    split_excess_waits(nc)

    common = {k: np.ascontiguousarray(np.asarray(inputs[k], np.float32))
              for k in ("W_x", "W_h", "W_k", "W_v", "W_lam", "b", "b_lam")}
    in_maps = []
    for c in range(N_CORES):
        m = dict(common)
        m["x"] = np.ascontiguousarray(x[c * Bl:(c + 1) * Bl])
        in_maps.append(m)
    import os
    trace = bool(int(os.environ.get("BASS_KERNEL_TRACE", "0")))
    kw = {}
    td = os.environ.get("BASS_KERNEL_TRACE_DIR")
    if trace and td:
        kw["tmpdir"] = td
    res = run_bass_kernel_spmd(nc, in_maps, list(range(N_CORES)), trace=trace, **kw)
    global LAST_EXEC_NS
    LAST_EXEC_NS = res.exec_time_ns
    outs = []
    for c in range(N_CORES):
        o = res.results[c]["out"]                      # (T, 128, 512)
        o = o.reshape(Tl, Hl // 512, 32, 512)[:, :, :Bl, :]
        outs.append(np.transpose(o, (2, 0, 1, 3)).reshape(Bl, Tl, Hl))
    return np.concatenate(outs, axis=0).astype(np.float32)


# revision 46
# speedup vs baseline: 1.0897x; 1.0897x over previous
"""DREAM rank-R fast-weight recurrence kernel for Trainium2 (8 NeuronCores).

Strategy (data-parallel over batch, 8 samples/core):
  Phase 1: transpose x -> x.T tiles (PE transposes); bf16 + fp32 copies.
  Phase A: v = x @ W_v  (big bf16 matmuls) -> DRAM.
  Phase B: chunked linear attention (chunk C=128) producing
           z = x@W_x + b + fast  (fast = decayed rank-R readout) -> DRAM fp32.
           x@W_x runs in float32r (tf32-ish) — same 1 cycle/row rate as bf16
           at N=512 but ~0.05% error instead of 0.4%.
  Phase C: sequential scan over T=512: h_t = tanh(z_t + h_{t-1} @ W_h),
           software-pipelined: body(iv) = [post-process step iv from psum
           slot iv%2 (tanh -> PE transpose -> pack), interleaved quarter-by-
           quarter with the matmul phase of step iv+1 into slot (iv+1)%2].
           This keeps the PE streaming continuously (HAM stays warm) and
           overlaps tanh/transpose/pack of one step with matmuls of the next.
Output written as (T, B, H) fp32; host just transposes to (B, T, H).
"""

import numpy as np

LAST_EXEC_NS = None

import concourse.bass as bass
import concourse.mybir as mybir
import concourse.tile as tile
from concourse.bass import _add_dep_helper
from concourse.masks import make_identity, make_upper_triangular

FP = mybir.dt.float32
FR = mybir.dt.float32r
BF = mybir.dt.bfloat16
ACTF = mybir.ActivationFunctionType
ALU = mybir.AluOpType

# full-problem config
B_FULL, T, I, H, R = 64, 512, 1024, 2048, 8
N_CORES = 8
SCAN_FR = False  # h @ W_h in float32r (else bf16; fp32r can't target offset
                 # psum partitions — s3d3_mm_valid_dst_partition)
ZX_FR = True     # x @ W_x in float32r (else bf16)


def _squeeze0(ap):
    # drop a leading size-1 dim from an AP via rearrange
    shape = ap.shape
    assert shape[0] == 1
    letters = list("abcdefg"[: len(shape)])
    lhs = " ".join(letters)
    rhs = f"({letters[0]} {letters[1]})" + ("" if len(letters) < 3 else " " + " ".join(letters[2:]))
    return ap.rearrange(f"{lhs} -> {rhs}")


def split_excess_waits(nc, max_waits=1):
    """walrus in this env supports only 1 sync-wait per instruction; split
    extras onto standalone NoOps on the same engine (order-preserving)."""
    n = 0
    for fn in nc.m.functions:
        for bb in fn.blocks:
            new_insts = []
            for inst in bb.instructions:
                si = getattr(inst, "sync_info", None)
                if si is not None and si.on_wait and len(si.on_wait) > max_waits:
                    waits = list(si.on_wait)
                    extra, keep = waits[:-max_waits], waits[-max_waits:]
                    for w in extra:
                        new_insts.append(mybir.InstNoOp(
                            name=f"waitsplit_{nc.next_id()}",
                            engine=inst.engine, ins=[], outs=[],
                            bass_nofuse=True,
                            sync_info=mybir.SyncInfo(on_wait=[w], on_update=[]),
                        ))
                        n += 1
                    inst.sync_info = mybir.SyncInfo(on_wait=keep, on_update=list(si.on_update))
                new_insts.append(inst)
            bb.instructions = new_insts
    return n


def build_kernel(nc, B=8, T=T, I=I, H=H, R=R, unroll=8):
    """Emit the full per-core program. B = per-core batch."""
    _chain = {}

    def _dep(tag, inst):
        cur_bb = nc.cur_bb.bb.name if nc.cur_bb is not None else None
        prev = _chain.get(tag)
        if prev is not None and prev[0] == cur_bb:
            _add_dep_helper(inst.ins, prev[1].ins, sync=False, reason="psum chain")
        _chain[tag] = (cur_bb, inst)
        return inst

    def cmm(tag, *a, **kw):
        return _dep(tag, nc.tensor.matmul(*a, **kw))

    def ctr(tag, *a, **kw):
        return _dep(tag, nc.tensor.transpose(*a, **kw))

    KI, KH = I // 128, H // 128
    NS = max(1, H // 512)          # 512-wide column slices of H
    assert H % 512 == 0
    NTOK = B * T
    NC_CH = T // 128               # chunks (chunk == 128-token window)
    assert B <= 8 and T % 128 == 0 and I % 128 == 0
    NQ = (B + 3) // 4              # F state tiles (4 batches per 128-partition tile)

    x_ext = nc.declare_dram_parameter("x", [B, T, I], FP, isOutput=False)
    Wx_ext = nc.declare_dram_parameter("W_x", [I, H], FP, isOutput=False)
    Wh_ext = nc.declare_dram_parameter("W_h", [H, H], FP, isOutput=False)
    Wk_ext = nc.declare_dram_parameter("W_k", [I, R], FP, isOutput=False)
    Wv_ext = nc.declare_dram_parameter("W_v", [I, H], FP, isOutput=False)
    Wl_ext = nc.declare_dram_parameter("W_lam", [I, R], FP, isOutput=False)
    b_ext = nc.declare_dram_parameter("b", [H], FP, isOutput=False)
    bl_ext = nc.declare_dram_parameter("b_lam", [R], FP, isOutput=False)
    out_ext = nc.declare_dram_parameter("out", [T, 128, 512], FP, isOutput=True)

    # z stored as two-term bf16 (hi at [0:H], lo residual at [H:2H]) so the
    # scan's identity-matmul injection stays bf16 while z keeps ~fp32
    # precision. Step dim padded so zbuf prefetch may read 2 steps past T.
    z_dram = nc.dram_tensor("z_dram", [B, T + 8, 2 * H], BF)

    ZXD = FR if ZX_FR else BF      # dtype of the x@W_x matmuls
    SCD = FR if SCAN_FR else BF    # dtype of the scan matmuls

    with tile.TileContext(nc) as tc:
        # ---------- persistent pool ----------
        with tc.tile_pool(name="persist", bufs=1) as pp:
            I128 = pp.tile([128, 128], FP)
            make_identity(nc, I128[:])
            I128b = pp.tile([128, 128], BF)
            nc.vector.tensor_copy(I128b[:], I128[:])
            I8rep = pp.tile([128, max(B, 1)], FP)
            nc.gpsimd.memset(I8rep[:], 0.0)
            for g in range(4):
                nc.gpsimd.affine_select(
                    out=I8rep[:], in_=I8rep[:], compare_op=ALU.not_equal,
                    fill=1.0, base=-32 * g, pattern=[[-1, B]], channel_multiplier=1)
            IB_bf = pp.tile([B, B], BF)
            nc.vector.tensor_copy(IB_bf[:], I8rep[0:B, 0:B])
            # interleaved double identity [2B, B] for one-shot hi+lo z
            # injection: rows 2j and 2j+1 both map to output row j
            IB2_f = pp.tile([2 * B, B], FP)
            nc.gpsimd.memset(IB2_f[:], 0.0)
            for off in range(2):
                nc.gpsimd.affine_select(
                    out=IB2_f[:], in_=IB2_f[:], compare_op=ALU.not_equal,
                    fill=1.0, base=-off, pattern=[[-2, B]], channel_multiplier=1)
            IB2_bf = pp.tile([2 * B, B], BF)
            nc.vector.tensor_copy(IB2_bf[:], IB2_f[:])
            Tri = pp.tile([128, 128], FP)       # Tri[u,t] = u <= t
            make_upper_triangular(nc, Tri[:], val=1.0, diag=True)
            STri = pp.tile([128, 128], FP)      # STri[u,t] = u < t
            make_upper_triangular(nc, STri[:], val=1.0, diag=False)

            b_bcast = pp.tile([128, H], FP)
            nc.sync.dma_start(out=b_bcast[:], in_=b_ext[None, :].to_broadcast((128, H)))
            bl_bcast = pp.tile([128, R], FP)
            nc.sync.dma_start(out=bl_bcast[:], in_=bl_ext[None, :].to_broadcast((128, R)))

            Wk_bf = pp.tile([128, KI, R], BF)
            Wl_bf = pp.tile([128, KI, R], BF)

            def load_xT(j, w, pool, pspool, tagp, want32=False):
                # DMA an x window (128 tok, I) and transpose to [128, KI, 128]
                xs = pool.tile([128, I], FP, name=f"xs_{tagp}_{j}_{w}", tag=f"xs_{tagp}", bufs=2)
                nc.sync.dma_start(out=xs[:], in_=x_ext[j, w * 128:(w + 1) * 128, :])
                xTw = pool.tile([128, KI, 128], BF, name=f"xTw_{tagp}_{j}_{w}", tag=f"xTw_{tagp}", bufs=2)
                xT32 = None
                if want32:
                    xT32 = pool.tile([128, KI, 128], FP, name=f"xT32_{tagp}_{j}_{w}", tag=f"xT32_{tagp}", bufs=2)
                for ki in range(KI):
                    xt_ps = pspool.tile([128, 128], FP, name=f"xtps_{tagp}_{j}_{w}_{ki}", tag=f"xtps_{tagp}", bufs=1)
                    ctr(f"xtps_{tagp}", xt_ps[:], xs[:, ki * 128:(ki + 1) * 128], I128[:])
                    nc.vector.tensor_copy(xTw[:, ki, :], xt_ps[:])
                    if want32:
                        nc.scalar.copy(xT32[:, ki, :].bitcast(FR), xt_ps[:])
                return xTw, xT32

            with tc.tile_pool(name="wstage", bufs=2) as wsp:
                for ki in range(KI):
                    wkst = wsp.tile([128, R], FP, name=f"wkst{ki}", tag="wkst")
                    nc.sync.dma_start(out=wkst[:], in_=Wk_ext[ki * 128:(ki + 1) * 128, :])
                    nc.vector.tensor_copy(Wk_bf[:, ki, :], wkst[:])
                    wlst = wsp.tile([128, R], FP, name=f"wlst{ki}", tag="wlst")
                    nc.sync.dma_start(out=wlst[:], in_=Wl_ext[ki * 128:(ki + 1) * 128, :])
                    nc.vector.tensor_copy(Wl_bf[:, ki, :], wlst[:])

            # ---------- Phase B: v = x@W_v inline; z = x@W_x + b + fast ----------
            with tc.tile_pool(name="phB", bufs=2) as pB, \
                 tc.tile_pool(name="phBst", bufs=1) as pBst, \
                 tc.tile_pool(name="phBps", bufs=2, space="PSUM") as pBps:
                Wv_bf = pBst.tile([128, KI, H], BF)
                for ki in range(KI):
                    wst = pB.tile([128, H], FP, name=f"wvst{ki}", tag="wvst", bufs=1)
                    nc.sync.dma_start(out=wst[:], in_=Wv_ext[ki * 128:(ki + 1) * 128, :])
                    nc.vector.tensor_copy(Wv_bf[:, ki, :], wst[:])
                if ZX_FR:
                    Wx_st = pBst.tile([128, KI, H], FP)
                    for ki in range(KI):
                        nc.sync.dma_start(out=Wx_st[:, ki, :].bitcast(FR),
                                          in_=Wx_ext[ki * 128:(ki + 1) * 128, :].bitcast(FR))
                else:
                    Wx_st = pBst.tile([128, KI, H], BF)
                    for ki in range(KI):
                        wst2 = pB.tile([128, H], FP, name=f"wxst{ki}", tag="wxst", bufs=1)
                        nc.sync.dma_start(out=wst2[:], in_=Wx_ext[ki * 128:(ki + 1) * 128, :])
                        nc.vector.tensor_copy(Wx_st[:, ki, :], wst2[:])
                F_T = [pBst.tile([128, H], FP, name=f"FT{q}") for q in range(NQ)]
                F_bfT = [pBst.tile([128, H], BF, name=f"FbfT{q}") for q in range(NQ)]
                for q in range(NQ):
                    nc.gpsimd.memset(F_T[q][:], 0.0)
                    nc.gpsimd.memset(F_bfT[q][:], 0.0)

                for c in range(NC_CH):
                    cc_all = [pB.tile([128, 1], FP, name=f"cc{c}_{q}", tag=f"cc{q}")
                              for q in range(NQ)]
                    for q in range(NQ):
                        nc.gpsimd.memset(cc_all[q][:], 0.0)
                    b_bfs, v_sbs = [], []
                    for j in range(B):
                        q, m = j // 4, j % 4
                        tok0 = j * T + c * 128
                        xTw, xT32 = load_xT(j, c, pB, pBps, "B", want32=ZX_FR)
                        # -- v chunk computed inline (no DRAM round trip);
                        # ki-outer pairs share each weight load
                        v_sb = pB.tile([128, H], BF, name=f"vst{c}_{j}", tag=f"vst{j}", bufs=1)
                        for half in range(2):
                            vps = [pBps.tile([128, 512], FP, name=f"vb{c}_{j}_{half}_{p}",
                                             tag=f"vbps{p}", bufs=1) for p in range(2)]
                            for ki in range(KI):
                                for p in range(2):
                                    s = 2 * half + p
                                    cmm(f"vbps{p}", vps[p][:], xTw[:, ki, :],
                                        Wv_bf[:, ki, s * 512:(s + 1) * 512],
                                        start=(ki == 0), stop=(ki == KI - 1))
                            for p in range(2):
                                s = 2 * half + p
                                nc.vector.tensor_copy(v_sb[:, s * 512:(s + 1) * 512], vps[p][:])
                        v_sbs.append(v_sb)
                        # -- k, lam projections (psum)
                        sm = pBps.tile([128, 512], FP, name=f"small{c}_{j}", tag="small", bufs=1)
                        k_ps, l_ps = sm[:, 0:R], sm[:, 16:16 + R]
                        Lc_ps, Lcp_ps = sm[:, 32:32 + R], sm[:, 48:48 + R]
                        aT_ps, bT_ps, LcT_ps = sm[:, 64:192], sm[:, 192:320], sm[:, 320:448]
                        for ki in range(KI):
                            cmm("small", k_ps, xTw[:, ki, :],
                                Wk_bf[:, ki, :], start=(ki == 0), stop=(ki == KI - 1))
                        for ki in range(KI):
                            cmm("small", l_ps, xTw[:, ki, :],
                                Wl_bf[:, ki, :], start=(ki == 0), stop=(ki == KI - 1))
                        # -- k = elu(.)+1 normalized
                        r1 = pB.tile([128, R], FP, name=f"r1{c}{j}", tag="r1")
                        nc.scalar.activation(r1[:], k_ps, ACTF.Relu)
                        r2 = pB.tile([128, R], FP, name=f"r2{c}{j}", tag="r2")
                        nc.vector.tensor_scalar_min(r2[:], k_ps, 0.0)
                        r3 = pB.tile([128, R], FP, name=f"r3{c}{j}", tag="r3")
                        nc.scalar.activation(r3[:], r2[:], ACTF.Exp)
                        ku = pB.tile([128, R], FP, name=f"ku{c}{j}", tag="ku")
                        nc.vector.tensor_tensor(ku[:], r1[:], r3[:], op=ALU.add)
                        ksum = pB.tile([128, 1], FP, name=f"ksum{c}{j}", tag="ksum")
                        nc.vector.reduce_sum(ksum[:], ku[:], axis=mybir.AxisListType.X)
                        kinv = pB.tile([128, 1], FP, name=f"kinv{c}{j}", tag="kinv")
                        nc.vector.reciprocal(kinv[:], ksum[:])
                        k_sb = pB.tile([128, R], FP, name=f"ksb{c}{j}", tag="ksb")
                        nc.vector.tensor_scalar_mul(k_sb[:], ku[:], kinv[:])
                        # -- loglam = -ln(1 + exp(-(zlam + b_lam)))
                        zl = pB.tile([128, R], FP, name=f"zl{c}{j}", tag="zl")
                        nc.vector.tensor_tensor(zl[:], l_ps, bl_bcast[:, :], op=ALU.add)
                        el = pB.tile([128, R], FP, name=f"el{c}{j}", tag="el")
                        nc.scalar.activation(el[:], zl[:], ACTF.Exp, scale=-1.0)
                        ep1 = pB.tile([128, R], FP, name=f"ep1{c}{j}", tag="ep1")
                        nc.vector.tensor_scalar_add(ep1[:], el[:], 1.0)
                        lsp = pB.tile([128, R], FP, name=f"lsp{c}{j}", tag="lsp")
                        nc.scalar.activation(lsp[:], ep1[:], ACTF.Ln)
                        ll = pB.tile([128, R], FP, name=f"ll{c}{j}", tag="ll")
                        nc.vector.tensor_scalar_mul(ll[:], lsp[:], -1.0)
                        # -- cumsums over time (within chunk) via triangular matmuls
                        cmm("small", Lc_ps, Tri[:], ll[:], start=True, stop=True)
                        cmm("small", Lcp_ps, STri[:], ll[:], start=True, stop=True)
                        cprev = pB.tile([128, R], FP, name=f"cprev{c}{j}", tag="cprev")
                        nc.scalar.activation(cprev[:], Lcp_ps, ACTF.Exp)
                        cinv = pB.tile([128, R], FP, name=f"cinv{c}{j}", tag="cinv")
                        nc.scalar.activation(cinv[:], Lc_ps, ACTF.Exp, scale=-1.0)
                        a_sb = pB.tile([128, R], FP, name=f"asb{c}{j}", tag="asb")
                        nc.vector.tensor_tensor(a_sb[:], k_sb[:], cprev[:], op=ALU.mult)
                        b_sb = pB.tile([128, R], FP, name=f"bsb{c}{j}", tag="bsb")
                        nc.vector.tensor_tensor(b_sb[:], k_sb[:], cinv[:], op=ALU.mult)
                        b_bf = pB.tile([128, R], BF, name=f"bbf{c}_{j}", tag=f"bbf{j}", bufs=2)
                        nc.vector.tensor_copy(b_bf[:], b_sb[:])
                        b_bfs.append(b_bf)
                        Lc_sb = pB.tile([128, R], FP, name=f"lcsb{c}{j}", tag="lcsb")
                        nc.vector.tensor_copy(Lc_sb[:], Lc_ps)
                        # -- transposes to (R, 128) at base 0 (walrus: transpose out base must be 0)
                        ctr("small", aT_ps[0:R, :], a_sb[:], I128[:])
                        ctr("small", bT_ps[0:R, :], b_sb[:], I128[:])
                        ctr("small", LcT_ps[0:R, :], Lc_sb[:], I128[:])
                        aT_bf = pB.tile([R, 128], BF, name=f"aT{c}{j}", tag="aT")
                        nc.vector.tensor_copy(aT_bf[:], aT_ps[0:R, :])
                        bT_bf = pB.tile([R, 128], BF, name=f"bT{c}{j}", tag="bT")
                        nc.vector.tensor_copy(bT_bf[:], bT_ps[0:R, :])
                        # replicate a.T to base 32m for the inter matmul (DMA crosses partitions)
                        aT32_bf = pB.tile([128, 128], BF, name=f"aT32_{c}{j}", tag="aT32")
                        nc.sync.dma_start(out=aT32_bf[32 * m:32 * m + R, :], in_=aT_bf[:])
                        cc0 = pB.tile([R, 1], FP, name=f"cc0{c}{j}", tag="cc0")
                        nc.scalar.activation(cc0[:], LcT_ps[0:R, 127:128], ACTF.Exp)
                        nc.sync.dma_start(out=cc_all[q][32 * m:32 * m + R, :], in_=cc0[:])
                        # -- S.T = mask(b @ a.T) (u,t), bf16
                        ST_ps = pBps.tile([128, 128], FP, name=f"stps{c}{j}", tag="stps", bufs=1)
                        cmm("stps", ST_ps[:], bT_bf[:], aT_bf[:], start=True, stop=True)
                        ST_sb = pB.tile([128, 128], BF, name=f"stsb{c}{j}", tag="stsb")
                        nc.vector.tensor_tensor(ST_sb[:], ST_ps[:], STri[:], op=ALU.mult)
                        # -- z tile: x@W_x + intra + inter, + bias on evac
                        for s in range(NS):
                            z_ps = pBps.tile([128, 512], FP, name=f"zps{c}_{j}_{s}", tag="zps", bufs=2)
                            for ki in range(KI):
                                if ZX_FR:
                                    cmm("zps", z_ps[:], xT32[:, ki, :].bitcast(FR),
                                        Wx_st[:, ki, s * 512:(s + 1) * 512].bitcast(FR),
                                        start=(ki == 0), stop=False)
                                else:
                                    cmm("zps", z_ps[:], xTw[:, ki, :],
                                        Wx_st[:, ki, s * 512:(s + 1) * 512],
                                        start=(ki == 0), stop=False)
                            cmm("zps", z_ps[:], ST_sb[:], v_sb[:, s * 512:(s + 1) * 512],
                                start=False, stop=(c == 0))
                            if c > 0:
                                cmm("zps", z_ps[:], aT32_bf[32 * m:32 * m + R, :],
                                    F_bfT[q][32 * m:32 * m + R, s * 512:(s + 1) * 512],
                                    start=False, stop=True,
                                    tile_position=(32 * m, 0))
                            z_sb = pB.tile([128, 512], FP, name=f"zsb{c}_{j}_{s}", tag="zsb")
                            nc.vector.tensor_tensor(z_sb[:], z_ps[:],
                                                    b_bcast[:, s * 512:(s + 1) * 512], op=ALU.add)
                            z_hi = pB.tile([128, 512], BF, name=f"zhi{c}_{j}_{s}", tag="zhi")
                            nc.scalar.copy(z_hi[:], z_sb[:])
                            z_lo = pB.tile([128, 512], BF, name=f"zlo{c}_{j}_{s}", tag="zlo")
                            nc.vector.tensor_tensor(z_lo[:], z_sb[:], z_hi[:],
                                                    op=ALU.subtract)
                            rows = slice(c * 128, (c + 1) * 128)
                            nc.sync.dma_start(
                                out=z_dram[j, rows, s * 512:(s + 1) * 512], in_=z_hi[:])
                            nc.sync.dma_start(
                                out=z_dram[j, rows, H + s * 512:H + (s + 1) * 512], in_=z_lo[:])
                    # -- F state update: F = (F + sum_u b_u v_u^T) * cC
                    for s in range(NS):
                        for q in range(NQ):
                            Fd_ps = pBps.tile([128, 512], FP, name=f"fd{c}_{s}_{q}",
                                              tag="fd", bufs=1)
                            nc.vector.memset(Fd_ps[:], 0.0)
                            for j in range(4 * q, min(4 * q + 4, B)):
                                m = j % 4
                                cmm("fd", Fd_ps[32 * m:32 * m + R, :], b_bfs[j][:],
                                    v_sbs[j][:, s * 512:(s + 1) * 512],
                                    start=True, stop=True,
                                    tile_position=(0, 32 * m))
                            sl = slice(s * 512, (s + 1) * 512)
                            nc.vector.tensor_tensor(F_T[q][:, sl], F_T[q][:, sl], Fd_ps[:],
                                                    op=ALU.add)
                            nc.vector.tensor_scalar_mul(F_T[q][:, sl], F_T[q][:, sl], cc_all[q][:])
                            nc.scalar.copy(F_bfT[q][:, sl], F_T[q][:, sl])

            # ---------- Phase C: the scan (software-pipelined) ----------
            with tc.tile_pool(name="phC", bufs=1) as pC, \
                 tc.tile_pool(name="phCps", bufs=1, space="PSUM") as pCps:
                HD = FP if SCAN_FR else BF      # h storage dtype
                if SCAN_FR:
                    Wh_st = pC.tile([128, KH, H], FP)
                    for kh in range(KH):
                        nc.sync.dma_start(out=Wh_st[:, kh, :].bitcast(FR),
                                          in_=Wh_ext[kh * 128:(kh + 1) * 128, :].bitcast(FR))
                else:
                    Wh_st = pC.tile([128, KH, H], BF)
                    with tc.tile_pool(name="whload", bufs=2) as wl:
                        for kh in range(KH):
                            wst3 = wl.tile([128, H], FP, name=f"whst{kh}", tag="whst")
                            nc.sync.dma_start(out=wst3[:], in_=Wh_ext[kh * 128:(kh + 1) * 128, :])
                            nc.vector.tensor_copy(Wh_st[:, kh, :], wst3[:])

                # packed transposed h: [128, 4(b)*4(a)*8(j)]; lhsT for tile
                # kh=(a,b) is h_T[:, 32b+8a : 32b+8a+8]
                assert not SCAN_FR
                h_T = pC.tile([128, 128], HD)
                nc.gpsimd.memset(h_T[:], 0.0)
                ps1 = [pCps.tile([128, 512], FP, name=f"ps1_{i}") for i in range(2)]
                ps2 = [pCps.tile([128, 128], HD, name=f"ps2_{i}") for i in range(2)]
                for i in range(2):
                    nc.vector.memset(ps1[i][:], 0.0)
                pre_t = [pC.tile([128, 512], HD, name=f"pret{i}") for i in range(2)]
                hout = [pC.tile([128, 512], FP, name=f"hout{i}") for i in range(2)]
                zbuf = [pC.tile([B, 2 * H], BF, name=f"zbuf{i}") for i in range(4)]

                def inject(slot, zb, stop=False):
                    # add z (hi + lo bf16 terms) into ps1[slot] via identity
                    # matmuls; hi terms start the accumulation group
                    for s in range(NS):
                        cmm("ps1", ps1[slot][32 * s:32 * s + B, :],
                            IB_bf[:], zb[:, 512 * s:512 * (s + 1)],
                            start=True, stop=False,
                            tile_position=(0, 32 * s), skip_group_check=True)
                    for s in range(NS):
                        cmm("ps1", ps1[slot][32 * s:32 * s + B, :],
                            IB_bf[:], zb[:, H + 512 * s:H + 512 * (s + 1)],
                            start=False, stop=stop and (s == NS - 1),
                            tile_position=(0, 32 * s), skip_group_check=True)

                def zdma(zb, iv_expr):
                    nc.sync.dma_start(
                        out=zb[:],
                        in_=_squeeze0(z_dram[0:B, iv_expr, :].rearrange("b one h -> one b h")))

                # prologue: prefetch z for steps 0/1, inject step 0 into slot 0
                zdma(zbuf[0], slice(0, 1))
                zdma(zbuf[1], slice(1, 2))
                inject(0, zbuf[0], stop=True)

                kcnt = [0]

                def step(iv):
                    k = kcnt[0] % unroll
                    kcnt[0] += 1
                    sl, nx = k % 2, (k + 1) % 2
                    # prefetch z for step iv+2
                    zdma(zbuf[(k + 2) % 4], bass.ds(iv + 2, 1))
                    # start accumulating step iv+1: z first
                    inject(nx, zbuf[(k + 1) % 4])
                    # post(iv) tanh quarters first (ACT runs them in order)
                    for b4 in range(4):
                        nc.scalar.activation(pre_t[sl][:, 128 * b4:128 * (b4 + 1)],
                                             ps1[sl][:, 128 * b4:128 * (b4 + 1)], ACTF.Tanh)

                    def trans(b4):
                        # PE transpose of quarter b4, streaming only the 32
                        # identity columns for valid (a, j) lanes, so the psum
                        # result lands pre-packed; DVE copy is then contiguous
                        ctr("ps2", ps2[sl][:, 32 * b4:32 * (b4 + 1)],
                            pre_t[sl][:, 128 * b4:128 * (b4 + 1)],
                            I128b[:].rearrange("p (a q) -> p a q", q=32)[:, :, 0:B])
                        nc.vector.tensor_copy(
                            h_T[:, 32 * b4:32 * (b4 + 1)],
                            ps2[sl][:, 32 * b4:32 * (b4 + 1)])

                    def burst(b4):
                        # matmul burst for quarter b4 of step iv+1
                        for a in range(4):
                            kh = 4 * a + b4
                            lhs = h_T[:, 32 * b4 + 8 * a:32 * b4 + 8 * a + 8]
                            for s in range(NS):
                                cmm("ps1", ps1[nx][32 * s:32 * s + B, :],
                                    lhs, Wh_st[:, kh, 512 * s:512 * (s + 1)],
                                    start=False, stop=(b4 == 3 and a == 3),
                                    tile_position=(0, 32 * s), skip_group_check=True)

                    # lookahead: PE does trans(b+1) while DVE packs quarter b,
                    # so each burst finds its lhsT ready without PE idle
                    trans(0)
                    for b4 in range(1, 4):
                        trans(b4)
                        burst(b4 - 1)
                    burst(3)
                    # fp32 output for step iv (off the critical path)
                    # fp32 output (garbage lanes included; host slices the
                    # valid 8-row band of each 32-partition group)
                    nc.scalar.activation(hout[sl][:], ps1[sl][:], ACTF.Tanh)
                    nc.sync.dma_start(out=_squeeze0(out_ext[bass.ds(iv, 1)]),
                                      in_=hout[sl][:])

                tc.For_i_unrolled(0, T, 1, step, max_unroll=unroll)

    return nc


# ---------------- host-side wrapper ----------------

def kernel(**inputs):
    from concourse.bass_utils import run_bass_kernel_spmd
    x = np.asarray(inputs["x"], np.float32)
    B, Tl, Il = x.shape
    Hl = np.asarray(inputs["W_x"]).shape[1]
    Rl = np.asarray(inputs["W_k"]).shape[1]
    Bl = B // N_CORES

    nc = bass.Bass()
    build_kernel(nc, B=Bl, T=Tl, I=Il, H=Hl, R=Rl)
    split_excess_waits(nc)

    common = {k: np.ascontiguousarray(np.asarray(inputs[k], np.float32))
              for k in ("W_x", "W_h", "W_k", "W_v", "W_lam", "b", "b_lam")}
    in_maps = []
    for c in range(N_CORES):
        m = dict(common)
        m["x"] = np.ascontiguousarray(x[c * Bl:(c + 1) * Bl])
        in_maps.append(m)
    import os
    trace = bool(int(os.environ.get("BASS_KERNEL_TRACE", "0")))
    kw = {}
    td = os.environ.get("BASS_KERNEL_TRACE_DIR")
    if trace and td:
        kw["tmpdir"] = td
    res = run_bass_kernel_spmd(nc, in_maps, list(range(N_CORES)), trace=trace, **kw)
    global LAST_EXEC_NS
    LAST_EXEC_NS = res.exec_time_ns
    outs = []
    for c in range(N_CORES):
        o = res.results[c]["out"]                      # (T, 128, 512)
        o = o.reshape(Tl, Hl // 512, 32, 512)[:, :, :Bl, :]
        outs.append(np.transpose(o, (2, 0, 1, 3)).reshape(Bl, Tl, Hl))
    return np.concatenate(outs, axis=0).astype(np.float32)
